# revision 40
# baseline (speedup 1.0000x reference)
"""nn_Attention — tensor-parallel causal attention on 8 TRN2 NeuronCores.

Contract: kernel(**inputs) takes the FULL unsharded inputs of the reference
(hidden_states (2,2048,2048) f32, c_attn_w (2048,6144), c_attn_b (6144,),
c_proj_w (2048,2048), c_proj_b (2048,)) and returns the full (2,2048,2048)
f32 output.

Sharding: batch x head-group tensor parallelism. Core c -> batch c//4,
head-group c%4 (4 of the 16 heads). Each core computes its QKV column slice,
causal attention for its heads, and a c_proj partial (rows slice); the host
gather sums the 4 partials per batch and adds the biases (c_proj_b plus the
exact v-bias contribution c_attn_b[2E:] @ c_proj_w, so no on-chip v bias).

Fused single-pass schedule (per core, bf16 matmuls, fp32 PSUM):
  for sc in 0..3:  p1(sc) -> attn(ci=sc, heads 0,1) -> proj(sc-1)
                   -> attn(ci=sc, heads 2,3)
  then proj(3). Interleaving the three stages spreads the ScalarE exp work
  (the attention-phase bottleneck) across PE-heavy QKV/proj sections.

  - x arrives pre-transposed and pre-cast to bf16 from the host (xT [E,S]),
    so phase 1 is plain DMA; the first chunk is split across three DMA
    queues (sync/scalar/gpsimd) to cut the startup stall,
  - p1: qT/kT = (Wqk_slice^T x^T) + b in transposed [j, s] layout (bias via
    ScalarE Identity-activation), v in natural [s, d] layout (swapped
    operands; PSUM->SBUF copies alternate ScalarE/VectorE),
  - attn: per query chunk, head-pair half-batches of (key-block, head)
    rounds; AV matmuls trail the score stream by AV_DELAY rounds to hide
    the ScalarE exp + VectorE mask latency. Causal diagonal blocks use
    narrowed moving operands. Softmax denominators: VectorE running adds
    of the exp tiles into a per-head fp16 accumulator (fp16 keeps the DVE
    in 2x mode and feeds the PE directly), then one ones-matmul per chunk
    broadcasts the cross-partition sum to all 128 partitions; fast
    reciprocal + multiply normalizes into outT,
  - proj: ec-outer / h-inner (one PSUM tile live), copies alternate
    ScalarE/VectorE into a per-row-block [128, 2048] staging tile, one
    output DMA per row block. The final chunk uses an h-split two-pass
    accumulation so its PE work starts before the last normalizes land.

  PSUM budget (8 banks): psum_main 2 (p1 groups, proj), psum_sc 3
  (score tiles + broadcast-sum), psum_out 3 (attention accumulators).
"""

import os
import sys

for _p in ("/opt/trn_rl_repo", "/root/.axon_site/_ro/trn_rl_repo"):
    if os.path.isdir(_p) and _p not in sys.path:
        sys.path.append(_p)

from contextlib import ExitStack

import numpy as np

import concourse.bass as bass
import concourse.tile as tile
from concourse import bacc, mybir
from concourse.bass_utils import run_bass_kernel_spmd
from concourse import bass_isa

F32 = mybir.dt.float32
BF16 = mybir.dt.bfloat16
FP16 = mybir.dt.float16
P = 128
CHUNK = 512
DIAG = CHUNK // P

S, E, NHEAD = 2048, 2048, 16
BATCH = 2
H = 4            # heads per core
NQK = 2 * H      # transposed-projection j-blocks (q,k only)
EB = E // P      # 16
SC = S // CHUNK  # 4
SB = S // P      # 16
EC = E // CHUNK  # 4
JB = S // P      # 16 key blocks per head
N_CORES = 8

def _emit(nc):
    scale = 1.0 / float(np.sqrt(P))

    xT = nc.dram_tensor("xT", [E, S], BF16, kind="ExternalInput").ap()
    wqj = nc.dram_tensor("wqj", [NQK * P, E], BF16, kind="ExternalInput").ap()
    wqv = nc.dram_tensor("wqv", [E, H * P], BF16, kind="ExternalInput").ap()
    bqk = nc.dram_tensor("bqk", [P, NQK], F32, kind="ExternalInput").ap()
    wproj = nc.dram_tensor("wproj", [H * P, E], BF16, kind="ExternalInput").ap()
    tri = nc.dram_tensor("tri", [P, 2 * CHUNK], BF16, kind="ExternalInput").ap()
    ones = nc.dram_tensor("ones", [P, P], FP16, kind="ExternalInput").ap()
    y = nc.dram_tensor("y", [S, E], F32, kind="ExternalOutput").ap()

    wqj_t = wqj.rearrange("(jb p) e -> jb p e", p=P)
    wqv_t = wqv.rearrange("(eb p) v -> eb p v", p=P)
    wproj_t = wproj.rearrange("(hb p) e -> hb p e", p=P)

    with tile.TileContext(nc) as tc, ExitStack() as ctx:
        const = ctx.enter_context(tc.tile_pool(name="const", bufs=1))
        wq_pool = ctx.enter_context(tc.tile_pool(name="wq", bufs=1))
        xt_pool = ctx.enter_context(tc.tile_pool(name="xt", bufs=1))
        qkvT_pool = ctx.enter_context(tc.tile_pool(name="qkvT", bufs=1))
        wp_pool = ctx.enter_context(tc.tile_pool(name="wp", bufs=1))
        outT_pool = ctx.enter_context(tc.tile_pool(name="outT", bufs=1))
        sum_pool = ctx.enter_context(tc.tile_pool(name="sum", bufs=1))
        exp_pool = ctx.enter_context(tc.tile_pool(name="exp", bufs=10))
        recip_pool = ctx.enter_context(tc.tile_pool(name="recip", bufs=2))
        yout_pool = ctx.enter_context(tc.tile_pool(name="yout", bufs=2))
        psum_main = ctx.enter_context(tc.tile_pool(name="psum_m", bufs=2, space="PSUM"))
        psum_sc = ctx.enter_context(tc.tile_pool(name="psum_s", bufs=2, space="PSUM"))
        psum_out = ctx.enter_context(tc.tile_pool(name="psum_o", bufs=2, space="PSUM"))

        # constants + first weight tile on the sync queue, then the first x
        # chunk split across both queues so the PE can start ~7us in.
        bq_t = const.tile([P, NQK], F32)
        nc.sync.dma_start(bq_t[:], bqk[:])
        tri_t = const.tile([P, 2 * CHUNK], BF16)
        nc.sync.dma_start(tri_t[:], tri[:])
        ones_t = const.tile([P, P], FP16)
        nc.sync.dma_start(ones_t[:], ones[:])

        wqj_tiles = [wq_pool.tile([P, E], BF16, name=f"wqj{jb}") for jb in range(NQK)]
        wqv_tiles = [
            wq_pool.tile([P, H * P], BF16, name=f"wqv{eb}") for eb in range(EB)
        ]
        xtb = [
            [xt_pool.tile([P, CHUNK], BF16, name=f"xt{b}_{eb}") for eb in range(EB)]
            for b in range(2)
        ]

        half_e = (EB // 2) * P
        nc.sync.dma_start(wqj_tiles[0][:, :half_e], wqj_t[0][:, :half_e])
        nc.scalar.dma_start(wqj_tiles[0][:, half_e:], wqj_t[0][:, half_e:])
        for eb in range(EB):
            q = (nc.sync, nc.scalar, nc.gpsimd)[eb % 3]
            q.dma_start(xtb[0][eb][:], xT[eb * P : (eb + 1) * P, 0:CHUNK])
        for jb in range(1, NQK):
            nc.sync.dma_start(wqj_tiles[jb][:], wqj_t[jb])
        for eb in range(EB):
            nc.sync.dma_start(wqv_tiles[eb][:], wqv_t[eb])
        wp_tiles = []
        for hb in range(H):
            t = wp_pool.tile([P, E], BF16, name=f"wp{hb}")
            nc.sync.dma_start(t[:], wproj_t[hb])
            wp_tiles.append(t)

        def load_chunk(sc):
            s0 = sc * CHUNK
            for eb in range(EB):
                nc.gpsimd.dma_start(
                    xtb[sc % 2][eb][:], xT[eb * P : (eb + 1) * P, s0 : s0 + CHUNK]
                )

        qkT = [qkvT_pool.tile([P, S], BF16, name=f"qkT{jb}") for jb in range(NQK)]
        vnat = [qkvT_pool.tile([P, H * P], BF16, name=f"vn{sb}") for sb in range(SB)]
        outT = [outT_pool.tile([P, S], BF16, name=f"outT{h}") for h in range(H)]
        sumacc = [sum_pool.tile([P, CHUNK], FP16, name=f"sa{h}") for h in range(H)]

        def emit_p1(sc):
            """QKV projection for sequence chunk sc."""
            if sc + 1 < SC:
                load_chunk(sc + 1)
            s0 = sc * CHUNK
            xt = xtb[sc % 2]
            for jb in range(NQK):
                ps = psum_main.tile([P, CHUNK], F32, name="ps_m")
                for eb in range(EB):
                    nc.tensor.matmul(
                        ps[:],
                        wqj_tiles[jb][:, eb * P : (eb + 1) * P],
                        xt[eb][:],
                        start=(eb == 0),
                        stop=(eb == EB - 1),
                    )
                nc.vector.tensor_scalar_add(
                    qkT[jb][:, s0 : s0 + CHUNK], ps[:], bq_t[:, jb : jb + 1]
                )
            for r in range(DIAG):
                sb = sc * DIAG + r
                ps = psum_main.tile([P, H * P], F32, name="ps_m")
                for eb in range(EB):
                    nc.tensor.matmul(
                        ps[:],
                        xt[eb][:, r * P : (r + 1) * P],
                        wqv_tiles[eb][:],
                        start=(eb == 0),
                        stop=(eb == EB - 1),
                    )
                nc.vector.tensor_copy(vnat[sb][:], ps[:])

        def emit_proj(pc, vector_only, split_h=False):
            """Output projection for the 4 row-blocks of query chunk pc.
            ec-outer so only one PSUM tile is live at a time. With split_h,
            each sb does an h0/h1 pass over all ec before h2/h3, so the
            final chunk's PE work can start before the last heads'
            normalize muls land (pp tiles from two pools, 4 live)."""
            for sb in range(4 * pc, 4 * pc + 4):
                ot = yout_pool.tile([P, E], F32, name="yo")
                if split_h:
                    pw = psum_sc.tile([P, 2 * CHUNK], F32, name="ps_s")
                    pp = [
                        pw[:, :CHUNK],
                        pw[:, CHUNK:],
                        psum_main.tile([P, CHUNK], F32, name="ps_m"),
                        psum_main.tile([P, CHUNK], F32, name="ps_m"),
                    ]
                    for h in range(H):
                        for ec in range(EC):
                            nc.tensor.matmul(
                                pp[ec],
                                outT[h][:, sb * P : (sb + 1) * P],
                                wp_tiles[h][:, ec * CHUNK : (ec + 1) * CHUNK],
                                start=(h == 0),
                                stop=(h == H - 1),
                                skip_group_check=True,
                            )
                    for ec in range(EC):
                        dst = ot[:, ec * CHUNK : (ec + 1) * CHUNK]
                        if (sb + ec) % 2 == 1:
                            nc.vector.tensor_copy(dst, pp[ec])
                        else:
                            nc.scalar.copy(dst, pp[ec])
                else:
                    for ec in range(EC):
                        pp = psum_main.tile([P, CHUNK], F32, name="ps_m")
                        for h in range(H):
                            nc.tensor.matmul(
                                pp[:],
                                outT[h][:, sb * P : (sb + 1) * P],
                                wp_tiles[h][:, ec * CHUNK : (ec + 1) * CHUNK],
                                start=(h == 0),
                                stop=(h == H - 1),
                            )
                        dst = ot[:, ec * CHUNK : (ec + 1) * CHUNK]
                        if vector_only or (sb + ec) % 2 == 1:
                            nc.vector.tensor_copy(dst, pp[:])
                        else:
                            nc.scalar.copy(dst, pp[:])
                nc.sync.dma_start(y[sb * P : (sb + 1) * P, :], ot[:])

        # ---- fused schedule: p1(sc) -> attn(ci=sc) in two head-pair halves
        # with proj(sc-1) spliced between them, so ScalarE exp work spreads
        # across PE-heavy sections and only 2 out_ps PSUM banks are held ----
        AV_DELAY = 8
        out_ps = {}

        def emit_av(sc, ent):
            h, jb, off, N, ex = ent
            jb_hi = 4 * sc + 3
            # running softmax-denominator accumulation (delayed with the AV
            # so the ordering with normalize reads stays correct); alternate
            # GpSimd/VectorE by key-block parity to split the add load
            if jb == 0:
                nc.vector.tensor_copy(sumacc[h][:, off:], ex)
            else:
                nc.vector.tensor_add(sumacc[h][:, off:], sumacc[h][:, off:], ex)
            if jb == 0:
                out_ps[h] = psum_out.tile([P, CHUNK], F32, name="ps_o")
            nc.tensor.matmul(
                out_ps[h][:, off:],
                vnat[jb][:, h * P : (h + 1) * P],
                ex,
                start=(jb == 0),
                stop=(jb == jb_hi),
                skip_group_check=True,
            )
            if jb == jb_hi:
                # chunk complete: broadcast-sum, reciprocal, normalize.
                # Mid-kernel batches reduce on the idle GpSimd engine (its
                # latency is hidden by the next p1 section); the final batch
                # keeps the PE ones-matmul path, which is latency-critical
                # during the flush.
                rc = recip_pool.tile([P, CHUNK], F32, name="rc")
                if sc < SC - 1:
                    rs = recip_pool.tile([P, CHUNK], F32, name="rc")
                    nc.gpsimd.partition_all_reduce(
                        rs[:], sumacc[h][:], channels=P,
                        reduce_op=bass_isa.ReduceOp.add,
                    )
                    nc.vector.reciprocal_approx_fast(rc[:], rs[:])
                else:
                    rc_ps = psum_sc.tile([P, 2 * CHUNK], F32, name="ps_s")
                    nc.tensor.matmul(
                        rc_ps[:, :CHUNK], ones_t[:], sumacc[h][:],
                        start=True, stop=True,
                    )
                    nc.vector.reciprocal_approx_fast(rc[:], rc_ps[:, :CHUNK])
                nc.vector.tensor_mul(
                    outT[h][:, sc * CHUNK : (sc + 1) * CHUNK],
                    out_ps.pop(h)[:],
                    rc[:],
                )

        def emit_attn_half(sc, h0):
            jb_hi = 4 * sc + 3
            trail = []  # delayed (h, jb, off, N, ex) awaiting AV
            for jb in range(jb_hi + 1):
                diag = jb >= 4 * sc
                off = (jb - 4 * sc) * P if diag else 0
                N = CHUNK - off
                # both heads' scores into halves (separate banks) of one
                # wide PSUM tile, then a single wide exp (and wide mask)
                sc_ps = psum_sc.tile([P, 2 * CHUNK], F32, name="ps_s")
                for i, h in enumerate((h0, h0 + 1)):
                    nc.tensor.matmul(
                        sc_ps[:, i * CHUNK : i * CHUNK + N],
                        qkT[H + h][:, jb * P : (jb + 1) * P],
                        qkT[h][:, sc * CHUNK + off : (sc + 1) * CHUNK],
                        start=True,
                        stop=True,
                        skip_group_check=True,
                    )
                exw = exp_pool.tile([P, 2 * CHUNK], BF16, name="ex")
                pair_ap = lambda t: t.rearrange("p (t c) -> p t c", t=2)[:, :, :N]
                nc.scalar.activation(
                    pair_ap(exw[:]),
                    pair_ap(sc_ps[:]),
                    mybir.ActivationFunctionType.Exp,
                    scale=scale,
                )
                if diag:
                    exm = exp_pool.tile([P, 2 * CHUNK], BF16, name="ex")
                    nc.vector.tensor_mul(
                        pair_ap(exm[:]), pair_ap(exw[:]), pair_ap(tri_t[:])
                    )
                    exw = exm
                for i, h in enumerate((h0, h0 + 1)):
                    ex = exw[:, i * CHUNK : i * CHUNK + N]
                    trail.append((h, jb, off, N, ex))
                    if len(trail) > AV_DELAY:
                        emit_av(sc, trail.pop(0))
            for ent in trail:
                emit_av(sc, ent)

        for sc in range(SC):
            emit_p1(sc)
            emit_attn_half(sc, 0)
            if sc >= 1:
                emit_proj(sc - 1, vector_only=True)
            emit_attn_half(sc, 2)
        emit_proj(SC - 1, vector_only=False, split_h=True)
    return nc


_NC = None
LAST_RESULTS = None


def _get_nc():
    global _NC
    if _NC is None:
        nc = bacc.Bacc(
            "TRN2", target_bir_lowering=False, debug=False, num_devices=N_CORES
        )
        _emit(nc)
        nc.compile()
        _NC = nc
    return _NC


def _prep_shared(hidden_states):
    """Per-batch xT (transposed, bf16) shared by the 4 cores of each batch."""
    import ml_dtypes

    bf16 = ml_dtypes.bfloat16
    return [
        np.ascontiguousarray(hidden_states[b].T).astype(bf16) for b in range(BATCH)
    ]


def _core_inputs(xTs, c_attn_w, c_attn_b, c_proj_w, core):
    import ml_dtypes

    bf16 = ml_dtypes.bfloat16
    b, g = core // 4, core % 4
    h0 = H * g
    qk_cols = []
    for part in range(2):
        for h in range(h0, h0 + H):
            base = part * E + h * P
            qk_cols.extend(range(base, base + P))
    qk_cols = np.asarray(qk_cols)
    # wqj[jb*P + k, eb*P + m] = W[eb*P + k, qk_col jb*P + m]
    wqk = np.ascontiguousarray(c_attn_w[:, qk_cols])  # [E, NQK*P]
    wqj = (
        wqk.reshape(EB, P, NQK, P).transpose(2, 1, 0, 3).reshape(NQK * P, E)
    ).astype(bf16)
    v_cols = np.arange(2 * E + h0 * P, 2 * E + (h0 + H) * P)
    wqv = np.ascontiguousarray(c_attn_w[:, v_cols]).astype(bf16)  # [E, H*P]
    bq = np.ascontiguousarray(c_attn_b[qk_cols]).astype(np.float32)
    bq = bq.reshape(NQK, P).T.copy()
    wproj = np.ascontiguousarray(c_proj_w[h0 * P : (h0 + H) * P, :]).astype(bf16)
    ii = np.arange(CHUNK)[None, :]
    pp = np.arange(P)[:, None]
    tri1 = (pp <= ii).astype(bf16)
    tri = np.concatenate([tri1, tri1], axis=1)
    ones = np.ones((P, P), dtype=np.float16)
    return {
        "xT": xTs[b],
        "wqj": wqj,
        "wqv": wqv,
        "bqk": bq,
        "wproj": wproj,
        "tri": tri,
        "ones": ones,
    }


def kernel(hidden_states, c_attn_w, c_attn_b, c_proj_w, c_proj_b):
    global LAST_RESULTS
    hidden_states = np.asarray(hidden_states)
    c_attn_w = np.asarray(c_attn_w)
    c_attn_b = np.asarray(c_attn_b)
    c_proj_w = np.asarray(c_proj_w)
    c_proj_b = np.asarray(c_proj_b)

    nc = _get_nc()
    xTs = _prep_shared(hidden_states)
    in_maps = [
        _core_inputs(xTs, c_attn_w, c_attn_b, c_proj_w, c) for c in range(N_CORES)
    ]
    res = run_bass_kernel_spmd(nc, in_maps, list(range(N_CORES)))
    LAST_RESULTS = res
    out = np.zeros((BATCH, S, E), dtype=np.float32)
    for c in range(N_CORES):
        out[c // 4] += res.results[c]["y"]
    # softmax weights sum to 1, so the v bias contributes exactly
    # c_attn_b[2E:] @ c_proj_w to every output row; fold it in with c_proj_b.
    bias = c_proj_b.astype(np.float64) + c_attn_b[2 * E :].astype(
        np.float64
    ) @ c_proj_w.astype(np.float64)
    out += bias.astype(np.float32)[None, None, :]
    return out
